# revision 82
# baseline (speedup 1.0000x reference)
"""NestedAttention Trainium2 kernel (fp8 DoubleRow mm2 + multi-engine softmax).

Reference computation (per batch b):
  q_i = wq[i] @ x ; k_j = wk[j] @ x ; v_j = wv[j] @ x        (1x1 convs, r=64)
  for i: acc_i = sum_j softmax_m(q_i^T k_j / sqrt(r)) applied to v_j
  out = wo @ concat_i(acc_i) ; y = x * sigmoid(out)

Sharding: 8 cores = batch(4) x query-column-halves(2). Each core holds full
k/v (m = 2304 keys) and a 1152-wide slice of query columns n; no cross-core
communication (softmax is over m, fully on-core).

Design (see git-less history in comments):
  * q/k/v projections are tiny input transforms -> computed on HOST (numpy)
    and DMA'd in pre-laid-out, freeing the PE/Act engines for attention.
  * mm1 (logits) in bf16: PE cost is column-count-bound either way.
  * exp() via Schraudolph bit-trick: q pre-scaled by 0.125*log2(e)*8, a spare
    contraction row (q_row64=56.25, k_row64=mask) adds the fp8 exponent bias,
    so PSUM holds the int8 BIT PATTERN of fp8e4m3(E). DVE-assigned m-tiles
    produce E with one f32->int8 convert-copy; Act-assigned tiles use true
    exp (scale ln2/8, mask=0). This splits softmax across both engines.
  * mm2 in fp8e4m3 MatmulPerfMode.DoubleRow: contracts 2x128 keys per pass
    (half the accumulation instructions). Stationary is [v^T | ones] so PSUM
    rows 0:64 = numerator, 64:128 = Z. Norm: shifted copy + recip + mul on
    DVE; j-sum folded into the final wo-projection's 9-term PSUM accumulation.
  * PSUM: mm1 out split [128,1024] (2 banks, 3 bufs) + grouped [128,512] tile
    holding four m-tiles' 128-col remainders (1 bank) + mm2 pa (1 bank).
    mm2 chunks are emitted interleaved at mm1 tiles 3/9/15 of the next pair
    so the 1-deep pa ring cycles without queue-order stalls.
"""

import os
import numpy as np

B, C, H, W = 4, 256, 48, 48
N = H * W            # 2304 keys (m) per image
NSLICE = N // 2      # 1152 query columns (n) per core
R = 64               # reduced channels
P = 128
MT = N // P          # 18 m-tiles
MT2 = MT // 2        # 9 double m-tiles for DoubleRow mm2
CHUNKS = [(0, 512), (512, 512), (1024, 128)]  # n chunks, PSUM-bank aligned
N_CORES = 8

LOG2E8 = 1.4426950408889634          # 0.125 * log2(e) * 8 folded into q
EXP_SCALE = 0.6931471805599453 / 8.0  # recovers exp(s/8) from 8*log2(E)
BIAS_BITS = 56.25                     # 8*(7 + c) Schraudolph bias constant

# E-production split: DVE takes whole 4-tile groups (uniform bias per group);
# Act (true exp) takes the rest. 8 DVE tiles = groups 1,3 = tiles 4-7, 12-15.
_ngrp = int(os.environ.get("NESTED_DVEGROUPS", "2"))
DVE_GROUPS = {0: (), 1: (1,), 2: (1, 3), 3: (0, 1, 3), 4: (0, 1, 2, 3)}[_ngrp]
DVE_SET = {4 * g + s for g in DVE_GROUPS for s in range(4)}
MM1PX = os.environ.get("NESTED_MM1PX", "0") == "1"  # fp8 DoublePixel mm1 expt
# all-Schraudolph mode: bias row everywhere, E via int8-convert on BOTH
# engines (Act does convert-copy too); ~11/7 Act:DVE big-part split
ACTI8 = os.environ.get("NESTED_ACTI8", "1") == "1"
ACTI8_DVE = {1, 4, 6, 9, 11, 14}

_CACHE = {}
LAST_RESULTS = None


def _build_program():
    from contextlib import ExitStack

    import concourse.bass as bass
    import concourse.tile as tile
    from concourse import bacc, mybir

    f32 = mybir.dt.float32
    bf16 = mybir.dt.bfloat16
    fp8 = mybir.dt.float8e4
    i8 = mybir.dt.int8
    Exp = mybir.ActivationFunctionType.Exp
    Sigmoid = mybir.ActivationFunctionType.Sigmoid
    mult = mybir.AluOpType.mult
    DR = mybir.MatmulPerfMode.DoubleRow
    PX = mybir.MatmulPerfMode.DoublePixel

    nc = bacc.Bacc("TRN2", target_bir_lowering=False, debug=False)
    qk_dt = fp8 if MM1PX else bf16
    q_d = nc.declare_dram_parameter("qh", [P, 3, NSLICE], qk_dt, isOutput=False)
    k_d = nc.declare_dram_parameter("kh", [P, 3, N], qk_dt, isOutput=False)
    vT_d = nc.declare_dram_parameter("vTh", [P, MT2, 2, 384], fp8, isOutput=False)
    xn_d = nc.declare_dram_parameter("xn", [2, P, NSLICE], f32, isOutput=False)
    woT_d = nc.declare_dram_parameter("woT", [3, R, C], bf16, isOutput=False)
    y_d = nc.declare_dram_parameter("y", [2, P, NSLICE], f32, isOutput=True)

    with tile.TileContext(nc) as tc, ExitStack() as ctx:
        consts = ctx.enter_context(tc.tile_pool(name="consts", bufs=1))
        # PSUM budget (8 banks): big 3x[128,1024]=6 + c3 1x[128,512]=1 + mm2 1
        big_ps = ctx.enter_context(tc.tile_pool(name="big_ps", bufs=3, space="PSUM"))
        c3_ps = ctx.enter_context(tc.tile_pool(name="c3_ps", bufs=1, space="PSUM"))
        mm2_ps = ctx.enter_context(tc.tile_pool(name="mm2_ps", bufs=1, space="PSUM"))
        e_pool = ctx.enter_context(tc.tile_pool(name="e_pool", bufs=3))
        rb_pool = ctx.enter_context(tc.tile_pool(name="rb_pool", bufs=2))
        small = ctx.enter_context(tc.tile_pool(name="small", bufs=4))

        # ---- persistent SBUF state (all host-prepared) ----
        # spread DMA issue across the three HWDGE engines so the first pair's
        # q/k land ASAP (Act/DVE are idle at t=0 anyway)
        q_sb = consts.tile([P, 3, NSLICE], qk_dt)
        k_sb = consts.tile([P, 3, N], qk_dt)
        nc.sync.dma_start(q_sb[:, 0, :], q_d[:, 0, :])
        nc.sync.dma_start(k_sb[:, 0, 0:NSLICE], k_d[:, 0, 0:NSLICE])
        nc.sync.dma_start(k_sb[:, 0, NSLICE:N], k_d[:, 0, NSLICE:N])
        for i in range(1, 3):
            nc.sync.dma_start(q_sb[:, i, :], q_d[:, i, :])
            nc.sync.dma_start(k_sb[:, i, 0:NSLICE], k_d[:, i, 0:NSLICE])
            nc.sync.dma_start(k_sb[:, i, NSLICE:N], k_d[:, i, NSLICE:N])
        vT_buf = consts.tile([P, MT2, 2, 384], fp8)
        nc.sync.dma_start(vT_buf[:], vT_d[:])
        xn_sb = consts.tile([P, 2, NSLICE], f32)
        nc.sync.dma_start(xn_sb[:], xn_d.rearrange("t p m -> p t m"))
        woT_sb = []
        for i in range(3):
            w = consts.tile([R, C], bf16, tag=f"woT{i}")
            nc.sync.dma_start(w[:], woT_d[i])
            woT_sb.append(w)

        # PE p-state warmup: dummy matmul chain runs during the input-DMA wait
        wu = consts.tile([P, 512], bf16, name="wu")
        nc.vector.memset(wu[:], 0.001)
        wups = mm2_ps.tile([P, 512], f32, tag="mm2", name="wups")
        for w in range(12):
            nc.tensor.matmul(
                wups[:, 0:512],
                wu[:, 0:128],
                wu[:, 0:512],
                start=(w == 0),
                stop=(w == 11),
            )

        # acc_ij in bf16; j-sum folded into final projection's PSUM accum
        acc = {}
        for i in range(3):
            for j in range(3):
                acc[(i, j)] = consts.tile(
                    [R, NSLICE], bf16, tag=f"acc{i}{j}", name=f"acc{i}{j}"
                )

        # ---- attention pair pipeline ----
        def emit_mm1_exp(i, j, pending):
            """mm1 + E for pair (i,j); fires pending mm2-chunk thunks of the
            previous pair at m-tiles 3/9/15 so they interleave in all queues."""
            E = e_pool.tile([P, MT2, 2, NSLICE], fp8, tag="E")
            c3_defer = []
            for mt in range(MT):
                g, slot = mt // 4, mt % 4
                pt = big_ps.tile([P, 1024], f32, tag="big", name=f"pt{mt}")
                for c0 in (0, 512):
                    nc.tensor.matmul(
                        pt[:, c0 : c0 + 512],
                        k_sb[:, j, mt * P : (mt + 1) * P],
                        q_sb[:, i, c0 : c0 + 512],
                        start=True,
                        stop=True,
                        perf_mode=PX if MM1PX else None,
                    )
                c3_defer.append(mt)
                if slot == 3 or mt == MT - 1:
                    # c3 tile allocated (and its matmuls emitted) only at the
                    # group's last tile: hides the 1-deep c3 ring wait that
                    # otherwise stalls the PE at every pair boundary
                    c3 = c3_ps.tile([P, 512], f32, tag="c3", name=f"c3g{g}")
                    for mtd in c3_defer:
                        nc.tensor.matmul(
                            c3[:, (mtd % 4) * 128 : (mtd % 4) * 128 + 128],
                            k_sb[:, j, mtd * P : (mtd + 1) * P],
                            q_sb[:, i, 1024:1152],
                            start=True,
                            stop=True,
                            perf_mode=PX if MM1PX else None,
                        )
                    c3_defer = []
                dst = E[:, mt // 2, mt % 2, 0:1024]
                if ACTI8:
                    if mt in ACTI8_DVE:
                        nc.vector.tensor_copy(dst.bitcast(i8), pt[:])
                    else:
                        nc.scalar.copy(dst.bitcast(i8), pt[:])
                elif mt in DVE_SET:
                    nc.vector.tensor_copy(dst.bitcast(i8), pt[:])
                else:
                    nc.scalar.activation(dst, pt[:], Exp, scale=EXP_SCALE)
                if slot == 3 or mt == MT - 1:
                    # grouped E for columns 1024:1152 of tiles 4g..mt
                    nslots = slot + 1
                    if nslots > 2:
                        bases = [c3[:, s * 128 : s * 128 + 128] for s in (0, 1)]
                        srcs = [
                            bass.AP(
                                tensor=b.tensor,
                                offset=b.offset,
                                ap=[b.ap[0], [256, nslots // 2], [1, 128]],
                            )
                            for b in bases
                        ]
                        dsts = [E[:, 2 * g : 2 * g + 2, s, 1024:1152] for s in (0, 1)]
                    else:
                        srcs = [
                            c3[:, 0 : nslots * 128].rearrange(
                                "p (s c) -> p s c", s=nslots
                            )
                        ]
                        dsts = [E[:, 2 * g, 0:nslots, 1024:1152]]
                    for si, (sdst, ssrc) in enumerate(zip(dsts, srcs)):
                        if ACTI8:
                            if (2 * g + si) % 3 == 2:
                                nc.vector.tensor_copy(sdst.bitcast(i8), ssrc)
                            else:
                                nc.scalar.copy(sdst.bitcast(i8), ssrc)
                        elif mt in DVE_SET:
                            nc.vector.tensor_copy(sdst.bitcast(i8), ssrc)
                        else:
                            nc.scalar.activation(sdst, ssrc, Exp, scale=EXP_SCALE)
                if mt in (1, 7, 13) and pending:
                    pending.pop(0)()
            return E

        po = [None, None]
        poc3 = [None]

        def emit_final_chunk(c0, cw):
            for mtile in range(2):
                if c0 < 1024:
                    out = po[mtile][:, c0 : c0 + cw]
                else:
                    out = poc3[0][:, mtile * 128 : mtile * 128 + cw]
                for i in range(3):
                    for j in range(3):
                        nc.tensor.matmul(
                            out,
                            woT_sb[i][:, mtile * P : (mtile + 1) * P],
                            acc[(i, j)][:, c0 : c0 + cw],
                            start=(i == 0 and j == 0),
                            stop=(i == 2 and j == 2),
                        )
            for mtile in range(2):
                if c0 < 1024:
                    src = po[mtile][:, c0 : c0 + cw]
                else:
                    src = poc3[0][:, mtile * 128 : mtile * 128 + cw]
                sig = small.tile([P, 512], f32, tag="sig")
                nc.scalar.activation(sig[:, 0:cw], src, Sigmoid)
                y_sb = small.tile([P, 512], f32, tag="ysb")
                nc.vector.tensor_tensor(
                    y_sb[:, 0:cw], xn_sb[:, mtile, c0 : c0 + cw], sig[:, 0:cw], mult
                )
                # alternate issue engines (= DMA queues) for the output drain
                eng = nc.sync if mtile == 0 else nc.scalar
                eng.dma_start(y_d[mtile][:, c0 : c0 + cw], y_sb[:, 0:cw])

        def mm2_chunk_thunks(i, j, E, last=False):
            def make(c0, cw):
                def thunk():
                    pa = mm2_ps.tile([P, 512], f32, tag="mm2")
                    for t in range(MT2):
                        nc.tensor.matmul(
                            pa[:, 0:cw],
                            vT_buf[:, t, :, 128 * j : 128 * (j + 1)],
                            E[:, t, :, c0 : c0 + cw],
                            start=(t == 0),
                            stop=(t == MT2 - 1),
                            perf_mode=DR,
                        )
                    rb = rb_pool.tile([R, 512], f32, tag="rb")
                    nc.vector.tensor_copy(rb[:, 0:cw], pa[R:P, 0:cw])
                    nc.vector.reciprocal_approx_fast(rb[:, 0:cw], rb[:, 0:cw])
                    nc.vector.tensor_tensor(
                        acc[(i, j)][:, c0 : c0 + cw], pa[0:R, 0:cw], rb[:, 0:cw], mult
                    )
                    if last:
                        emit_final_chunk(c0, cw)

                return thunk

            return [make(c0, cw) for c0, cw in CHUNKS]

        pairs = [(i, j) for j in range(3) for i in range(3)]
        pending = []
        for i, j in pairs:
            E = emit_mm1_exp(i, j, pending)
            assert not pending
            pending = mm2_chunk_thunks(i, j, E)
        po[0] = big_ps.tile([P, 1024], f32, tag="big", name="po0")
        po[1] = big_ps.tile([P, 1024], f32, tag="big", name="po1")
        poc3[0] = c3_ps.tile([P, 512], f32, tag="c3", name="poc3")
        pending = mm2_chunk_thunks(*pairs[-1], E, last=True)
        while pending:
            pending.pop(0)()

    nc.compile()
    return nc


def _get_program():
    if "nc" not in _CACHE:
        _CACHE["nc"] = _build_program()
    return _CACHE["nc"]


def _host_prep(x, wq, wk, wv, wo):
    import ml_dtypes

    bf16 = ml_dtypes.bfloat16
    fp8 = ml_dtypes.float8_e4m3fn
    qk_np = fp8 if MM1PX else bf16
    xf = np.asarray(x, np.float32).reshape(B, C, N)
    wq32 = np.asarray(wq, np.float32) * LOG2E8
    wk32 = np.asarray(wk, np.float32)
    wv32 = np.asarray(wv, np.float32)
    # projections on host: [B, 3, R, N]
    q_all = np.einsum("irc,bcn->birn", wq32, xf)
    k_all = np.einsum("irc,bcn->birn", wk32, xf)
    v_all = np.einsum("irc,bcn->birn", wv32, xf)

    # k bias row: 1.0 for Schraudolph m-tiles, 0.0 for Act true-exp tiles
    mask = np.zeros(N, np.float32)
    if ACTI8:
        mask[:] = 1.0
    else:
        for mt in DVE_SET:
            mask[mt * P : (mt + 1) * P] = 1.0

    # vT layout [p, t, s, 128j+r] = v[j, r, 128(2t+s)+p], ones at +64..128
    def vt_layout(vb):  # vb [3, R, N]
        vt = np.ones((P, MT2, 2, 3, P), np.float32)
        # [3, R, 9, 2, 128] -> [128(p), 9, 2, 3, 64]
        vt[:, :, :, :, 0:R] = vb.reshape(3, R, MT2, 2, P).transpose(4, 2, 3, 0, 1)
        return vt.reshape(P, MT2, 2, 384).astype(fp8)

    woT = np.ascontiguousarray(
        np.stack(
            [np.asarray(wo, np.float32)[:, R * i : R * (i + 1)].T for i in range(3)]
        )
    ).astype(bf16)

    in_maps = []
    for core in range(N_CORES):
        b, h = core // 2, core % 2
        qh = np.zeros((P, 3, NSLICE), np.float32)
        qh[0:R] = q_all[b, :, :, h * NSLICE : (h + 1) * NSLICE].transpose(1, 0, 2)
        qh[R] = BIAS_BITS
        kh = np.zeros((P, 3, N), np.float32)
        kh[0:R] = k_all[b].transpose(1, 0, 2)
        kh[R] = mask[None, :]
        xn32 = np.ascontiguousarray(
            xf[b].reshape(2, P, N)[:, :, h * NSLICE : (h + 1) * NSLICE]
        )
        in_maps.append(
            {
                "qh": qh.astype(qk_np),
                "kh": kh.astype(qk_np),
                "vTh": vt_layout(v_all[b]),
                "xn": xn32,
                "woT": woT,
            }
        )
    return in_maps


def kernel(x, wq, wk, wv, wo):
    global LAST_RESULTS
    from concourse.bass_utils import run_bass_kernel_spmd

    x = np.asarray(x)
    nc = _get_program()
    in_maps = _host_prep(
        x, np.asarray(wq), np.asarray(wk), np.asarray(wv), np.asarray(wo)
    )
    res = run_bass_kernel_spmd(nc, in_maps, core_ids=list(range(N_CORES)))
    LAST_RESULTS = res
    out = np.empty((B, C, N), np.float32)
    for core in range(N_CORES):
        b, h = core // 2, core % 2
        out[b][:, h * NSLICE : (h + 1) * NSLICE] = res.results[core]["y"].reshape(
            C, NSLICE
        )
    return out.reshape(B, C, H, W).astype(x.dtype, copy=False)


# revision 83
# speedup vs baseline: 1.0456x; 1.0456x over previous
"""NestedAttention Trainium2 kernel (fp8 DoubleRow mm2 + multi-engine softmax).

Reference computation (per batch b):
  q_i = wq[i] @ x ; k_j = wk[j] @ x ; v_j = wv[j] @ x        (1x1 convs, r=64)
  for i: acc_i = sum_j softmax_m(q_i^T k_j / sqrt(r)) applied to v_j
  out = wo @ concat_i(acc_i) ; y = x * sigmoid(out)

Sharding: 8 cores = batch(4) x query-column-halves(2). Each core holds full
k/v (m = 2304 keys) and a 1152-wide slice of query columns n; no cross-core
communication (softmax is over m, fully on-core).

Design (see git-less history in comments):
  * q/k/v projections are tiny input transforms -> computed on HOST (numpy)
    and DMA'd in pre-laid-out, freeing the PE/Act engines for attention.
  * mm1 (logits) in bf16: PE cost is column-count-bound either way.
  * exp() via Schraudolph bit-trick: q pre-scaled by 0.125*log2(e)*8, a spare
    contraction row (q_row64=56.25, k_row64=mask) adds the fp8 exponent bias,
    so PSUM holds the int8 BIT PATTERN of fp8e4m3(E). DVE-assigned m-tiles
    produce E with one f32->int8 convert-copy; Act-assigned tiles use true
    exp (scale ln2/8, mask=0). This splits softmax across both engines.
  * mm2 in fp8e4m3 MatmulPerfMode.DoubleRow: contracts 2x128 keys per pass
    (half the accumulation instructions). Stationary is [v^T | ones] so PSUM
    rows 0:64 = numerator, 64:128 = Z. Norm: shifted copy + recip + mul on
    DVE; j-sum folded into the final wo-projection's 9-term PSUM accumulation.
  * PSUM: mm1 out split [128,1024] (2 banks, 3 bufs) + grouped [128,512] tile
    holding four m-tiles' 128-col remainders (1 bank) + mm2 pa (1 bank).
    mm2 chunks are emitted interleaved at mm1 tiles 3/9/15 of the next pair
    so the 1-deep pa ring cycles without queue-order stalls.
"""

import os
import numpy as np

B, C, H, W = 4, 256, 48, 48
N = H * W            # 2304 keys (m) per image
NSLICE = N // 2      # 1152 query columns (n) per core
R = 64               # reduced channels
P = 128
MT = N // P          # 18 m-tiles
MT2 = MT // 2        # 9 double m-tiles for DoubleRow mm2
CHUNKS = [(0, 512), (512, 512), (1024, 128)]  # n chunks, PSUM-bank aligned
N_CORES = 8

LOG2E8 = 1.4426950408889634          # 0.125 * log2(e) * 8 folded into q
EXP_SCALE = 0.6931471805599453 / 8.0  # recovers exp(s/8) from 8*log2(E)
BIAS_BITS = 56.25                     # 8*(7 + c) Schraudolph bias constant

# E-production split: DVE takes whole 4-tile groups (uniform bias per group);
# Act (true exp) takes the rest. 8 DVE tiles = groups 1,3 = tiles 4-7, 12-15.
_ngrp = int(os.environ.get("NESTED_DVEGROUPS", "2"))
DVE_GROUPS = {0: (), 1: (1,), 2: (1, 3), 3: (0, 1, 3), 4: (0, 1, 2, 3)}[_ngrp]
DVE_SET = {4 * g + s for g in DVE_GROUPS for s in range(4)}
MM1PX = os.environ.get("NESTED_MM1PX", "0") == "1"  # fp8 DoublePixel mm1 expt
# all-Schraudolph mode: bias row everywhere, E via int8-convert on BOTH
# engines (Act does convert-copy too); ~11/7 Act:DVE big-part split
ACTI8 = os.environ.get("NESTED_ACTI8", "1") == "1"
ACTI8_DVE = {1, 4, 6, 9, 11, 14, 16}

_CACHE = {}
LAST_RESULTS = None


def _build_program():
    from contextlib import ExitStack

    import concourse.bass as bass
    import concourse.tile as tile
    from concourse import bacc, mybir

    f32 = mybir.dt.float32
    bf16 = mybir.dt.bfloat16
    fp8 = mybir.dt.float8e4
    i8 = mybir.dt.int8
    Exp = mybir.ActivationFunctionType.Exp
    Sigmoid = mybir.ActivationFunctionType.Sigmoid
    mult = mybir.AluOpType.mult
    DR = mybir.MatmulPerfMode.DoubleRow
    PX = mybir.MatmulPerfMode.DoublePixel

    nc = bacc.Bacc("TRN2", target_bir_lowering=False, debug=False)
    qk_dt = fp8 if MM1PX else bf16
    q_d = nc.declare_dram_parameter("qh", [P, 3, NSLICE], qk_dt, isOutput=False)
    k_d = nc.declare_dram_parameter("kh", [P, 3, N], qk_dt, isOutput=False)
    vT_d = nc.declare_dram_parameter("vTh", [P, MT2, 2, 384], fp8, isOutput=False)
    xn_d = nc.declare_dram_parameter("xn", [2, P, NSLICE], f32, isOutput=False)
    woT_d = nc.declare_dram_parameter("woT", [3, R, C], bf16, isOutput=False)
    y_d = nc.declare_dram_parameter("y", [2, P, NSLICE], f32, isOutput=True)

    with tile.TileContext(nc) as tc, ExitStack() as ctx:
        consts = ctx.enter_context(tc.tile_pool(name="consts", bufs=1))
        # PSUM budget (8 banks): big 3x[128,1024]=6 + c3 1x[128,512]=1 + mm2 1
        big_ps = ctx.enter_context(tc.tile_pool(name="big_ps", bufs=3, space="PSUM"))
        c3_ps = ctx.enter_context(tc.tile_pool(name="c3_ps", bufs=1, space="PSUM"))
        mm2_ps = ctx.enter_context(tc.tile_pool(name="mm2_ps", bufs=1, space="PSUM"))
        e_pool = ctx.enter_context(tc.tile_pool(name="e_pool", bufs=3))
        rb_pool = ctx.enter_context(tc.tile_pool(name="rb_pool", bufs=2))
        small = ctx.enter_context(tc.tile_pool(name="small", bufs=4))

        # ---- persistent SBUF state (all host-prepared) ----
        # spread DMA issue across the three HWDGE engines so the first pair's
        # q/k land ASAP (Act/DVE are idle at t=0 anyway)
        q_sb = consts.tile([P, 3, NSLICE], qk_dt)
        k_sb = consts.tile([P, 3, N], qk_dt)
        nc.sync.dma_start(q_sb[:, 0, :], q_d[:, 0, :])
        nc.sync.dma_start(k_sb[:, 0, 0:NSLICE], k_d[:, 0, 0:NSLICE])
        nc.sync.dma_start(k_sb[:, 0, NSLICE:N], k_d[:, 0, NSLICE:N])
        for i in range(1, 3):
            nc.sync.dma_start(q_sb[:, i, :], q_d[:, i, :])
            nc.sync.dma_start(k_sb[:, i, 0:NSLICE], k_d[:, i, 0:NSLICE])
            nc.sync.dma_start(k_sb[:, i, NSLICE:N], k_d[:, i, NSLICE:N])
        vT_buf = consts.tile([P, MT2, 2, 384], fp8)
        nc.sync.dma_start(vT_buf[:], vT_d[:])
        xn_sb = consts.tile([P, 2, NSLICE], f32)
        nc.sync.dma_start(xn_sb[:], xn_d.rearrange("t p m -> p t m"))
        woT_sb = []
        for i in range(3):
            w = consts.tile([R, C], bf16, tag=f"woT{i}")
            nc.sync.dma_start(w[:], woT_d[i])
            woT_sb.append(w)

        # PE p-state warmup: dummy matmul chain runs during the input-DMA wait
        wu = consts.tile([P, 512], bf16, name="wu")
        nc.vector.memset(wu[:], 0.001)
        wups = mm2_ps.tile([P, 512], f32, tag="mm2", name="wups")
        for w in range(12):
            nc.tensor.matmul(
                wups[:, 0:512],
                wu[:, 0:128],
                wu[:, 0:512],
                start=(w == 0),
                stop=(w == 11),
            )

        # acc_ij in bf16; j-sum folded into final projection's PSUM accum
        acc = {}
        for i in range(3):
            for j in range(3):
                acc[(i, j)] = consts.tile(
                    [R, NSLICE], bf16, tag=f"acc{i}{j}", name=f"acc{i}{j}"
                )

        # ---- attention pair pipeline ----
        def emit_mm1_exp(i, j, pending):
            """mm1 + E for pair (i,j); fires pending mm2-chunk thunks of the
            previous pair at m-tiles 3/9/15 so they interleave in all queues."""
            E = e_pool.tile([P, MT2, 2, NSLICE], fp8, tag="E")
            c3_defer = []
            for mt in range(MT):
                g, slot = mt // 4, mt % 4
                pt = big_ps.tile([P, 1024], f32, tag="big", name=f"pt{mt}")
                for c0 in (0, 512):
                    nc.tensor.matmul(
                        pt[:, c0 : c0 + 512],
                        k_sb[:, j, mt * P : (mt + 1) * P],
                        q_sb[:, i, c0 : c0 + 512],
                        start=True,
                        stop=True,
                        perf_mode=PX if MM1PX else None,
                    )
                c3_defer.append(mt)
                if slot == 3 or mt == MT - 1:
                    # c3 tile allocated (and its matmuls emitted) only at the
                    # group's last tile: hides the 1-deep c3 ring wait that
                    # otherwise stalls the PE at every pair boundary
                    c3 = c3_ps.tile([P, 512], f32, tag="c3", name=f"c3g{g}")
                    for mtd in c3_defer:
                        nc.tensor.matmul(
                            c3[:, (mtd % 4) * 128 : (mtd % 4) * 128 + 128],
                            k_sb[:, j, mtd * P : (mtd + 1) * P],
                            q_sb[:, i, 1024:1152],
                            start=True,
                            stop=True,
                            perf_mode=PX if MM1PX else None,
                        )
                    c3_defer = []
                dst = E[:, mt // 2, mt % 2, 0:1024]
                if ACTI8:
                    if mt in ACTI8_DVE:
                        nc.vector.tensor_copy(dst.bitcast(i8), pt[:])
                    else:
                        nc.scalar.copy(dst.bitcast(i8), pt[:])
                elif mt in DVE_SET:
                    nc.vector.tensor_copy(dst.bitcast(i8), pt[:])
                else:
                    nc.scalar.activation(dst, pt[:], Exp, scale=EXP_SCALE)
                if slot == 3 or mt == MT - 1:
                    # grouped E for columns 1024:1152 of tiles 4g..mt
                    nslots = slot + 1
                    if nslots > 2:
                        bases = [c3[:, s * 128 : s * 128 + 128] for s in (0, 1)]
                        srcs = [
                            bass.AP(
                                tensor=b.tensor,
                                offset=b.offset,
                                ap=[b.ap[0], [256, nslots // 2], [1, 128]],
                            )
                            for b in bases
                        ]
                        dsts = [E[:, 2 * g : 2 * g + 2, s, 1024:1152] for s in (0, 1)]
                    else:
                        srcs = [
                            c3[:, 0 : nslots * 128].rearrange(
                                "p (s c) -> p s c", s=nslots
                            )
                        ]
                        dsts = [E[:, 2 * g, 0:nslots, 1024:1152]]
                    for si, (sdst, ssrc) in enumerate(zip(dsts, srcs)):
                        if ACTI8:
                            if (2 * g + si) % 3 == 2:
                                nc.vector.tensor_copy(sdst.bitcast(i8), ssrc)
                            else:
                                nc.scalar.copy(sdst.bitcast(i8), ssrc)
                        elif mt in DVE_SET:
                            nc.vector.tensor_copy(sdst.bitcast(i8), ssrc)
                        else:
                            nc.scalar.activation(sdst, ssrc, Exp, scale=EXP_SCALE)
                if mt in (1, 7, 13) and pending:
                    pending.pop(0)()
            return E

        po = [None, None]
        poc3 = [None]

        def emit_final_chunk(c0, cw):
            for mtile in range(2):
                if c0 < 1024:
                    out = po[mtile][:, c0 : c0 + cw]
                else:
                    out = poc3[0][:, mtile * 128 : mtile * 128 + cw]
                for i in range(3):
                    for j in range(3):
                        nc.tensor.matmul(
                            out,
                            woT_sb[i][:, mtile * P : (mtile + 1) * P],
                            acc[(i, j)][:, c0 : c0 + cw],
                            start=(i == 0 and j == 0),
                            stop=(i == 2 and j == 2),
                        )
            for mtile in range(2):
                if c0 < 1024:
                    src = po[mtile][:, c0 : c0 + cw]
                else:
                    src = poc3[0][:, mtile * 128 : mtile * 128 + cw]
                sig = small.tile([P, 512], f32, tag="sig")
                nc.scalar.activation(sig[:, 0:cw], src, Sigmoid)
                y_sb = small.tile([P, 512], f32, tag="ysb")
                nc.vector.tensor_tensor(
                    y_sb[:, 0:cw], xn_sb[:, mtile, c0 : c0 + cw], sig[:, 0:cw], mult
                )
                # alternate issue engines (= DMA queues) for the output drain
                eng = nc.sync if mtile == 0 else nc.scalar
                eng.dma_start(y_d[mtile][:, c0 : c0 + cw], y_sb[:, 0:cw])

        def mm2_chunk_thunks(i, j, E, last=False):
            def make(c0, cw):
                def thunk():
                    pa = mm2_ps.tile([P, 512], f32, tag="mm2")
                    for t in range(MT2):
                        nc.tensor.matmul(
                            pa[:, 0:cw],
                            vT_buf[:, t, :, 128 * j : 128 * (j + 1)],
                            E[:, t, :, c0 : c0 + cw],
                            start=(t == 0),
                            stop=(t == MT2 - 1),
                            perf_mode=DR,
                        )
                    rb = rb_pool.tile([R, 512], f32, tag="rb")
                    nc.vector.tensor_copy(rb[:, 0:cw], pa[R:P, 0:cw])
                    nc.vector.reciprocal_approx_fast(rb[:, 0:cw], rb[:, 0:cw])
                    nc.vector.tensor_tensor(
                        acc[(i, j)][:, c0 : c0 + cw], pa[0:R, 0:cw], rb[:, 0:cw], mult
                    )
                    if last:
                        emit_final_chunk(c0, cw)

                return thunk

            return [make(c0, cw) for c0, cw in CHUNKS]

        pairs = [(i, j) for j in range(3) for i in range(3)]
        pending = []
        for i, j in pairs:
            E = emit_mm1_exp(i, j, pending)
            assert not pending
            pending = mm2_chunk_thunks(i, j, E)
        po[0] = big_ps.tile([P, 1024], f32, tag="big", name="po0")
        po[1] = big_ps.tile([P, 1024], f32, tag="big", name="po1")
        poc3[0] = c3_ps.tile([P, 512], f32, tag="c3", name="poc3")
        pending = mm2_chunk_thunks(*pairs[-1], E, last=True)
        while pending:
            pending.pop(0)()

    nc.compile()
    return nc


def _get_program():
    if "nc" not in _CACHE:
        _CACHE["nc"] = _build_program()
    return _CACHE["nc"]


def _host_prep(x, wq, wk, wv, wo):
    import ml_dtypes

    bf16 = ml_dtypes.bfloat16
    fp8 = ml_dtypes.float8_e4m3fn
    qk_np = fp8 if MM1PX else bf16
    xf = np.asarray(x, np.float32).reshape(B, C, N)
    wq32 = np.asarray(wq, np.float32) * LOG2E8
    wk32 = np.asarray(wk, np.float32)
    wv32 = np.asarray(wv, np.float32)
    # projections on host: [B, 3, R, N]
    q_all = np.einsum("irc,bcn->birn", wq32, xf)
    k_all = np.einsum("irc,bcn->birn", wk32, xf)
    v_all = np.einsum("irc,bcn->birn", wv32, xf)

    # k bias row: 1.0 for Schraudolph m-tiles, 0.0 for Act true-exp tiles
    mask = np.zeros(N, np.float32)
    if ACTI8:
        mask[:] = 1.0
    else:
        for mt in DVE_SET:
            mask[mt * P : (mt + 1) * P] = 1.0

    # vT layout [p, t, s, 128j+r] = v[j, r, 128(2t+s)+p], ones at +64..128
    def vt_layout(vb):  # vb [3, R, N]
        vt = np.ones((P, MT2, 2, 3, P), np.float32)
        # [3, R, 9, 2, 128] -> [128(p), 9, 2, 3, 64]
        vt[:, :, :, :, 0:R] = vb.reshape(3, R, MT2, 2, P).transpose(4, 2, 3, 0, 1)
        return vt.reshape(P, MT2, 2, 384).astype(fp8)

    woT = np.ascontiguousarray(
        np.stack(
            [np.asarray(wo, np.float32)[:, R * i : R * (i + 1)].T for i in range(3)]
        )
    ).astype(bf16)

    in_maps = []
    for core in range(N_CORES):
        b, h = core // 2, core % 2
        qh = np.zeros((P, 3, NSLICE), np.float32)
        qh[0:R] = q_all[b, :, :, h * NSLICE : (h + 1) * NSLICE].transpose(1, 0, 2)
        qh[R] = BIAS_BITS
        kh = np.zeros((P, 3, N), np.float32)
        kh[0:R] = k_all[b].transpose(1, 0, 2)
        kh[R] = mask[None, :]
        xn32 = np.ascontiguousarray(
            xf[b].reshape(2, P, N)[:, :, h * NSLICE : (h + 1) * NSLICE]
        )
        in_maps.append(
            {
                "qh": qh.astype(qk_np),
                "kh": kh.astype(qk_np),
                "vTh": vt_layout(v_all[b]),
                "xn": xn32,
                "woT": woT,
            }
        )
    return in_maps


def kernel(x, wq, wk, wv, wo):
    global LAST_RESULTS
    from concourse.bass_utils import run_bass_kernel_spmd

    x = np.asarray(x)
    nc = _get_program()
    in_maps = _host_prep(
        x, np.asarray(wq), np.asarray(wk), np.asarray(wv), np.asarray(wo)
    )
    res = run_bass_kernel_spmd(nc, in_maps, core_ids=list(range(N_CORES)))
    LAST_RESULTS = res
    out = np.empty((B, C, N), np.float32)
    for core in range(N_CORES):
        b, h = core // 2, core % 2
        out[b][:, h * NSLICE : (h + 1) * NSLICE] = res.results[core]["y"].reshape(
            C, NSLICE
        )
    return out.reshape(B, C, H, W).astype(x.dtype, copy=False)
